# revision 5
# baseline (speedup 1.0000x reference)
"""DPB (dynamic position bias) window attention kernel for Trainium2.

Contract: kernel(**inputs) takes the FULL unsharded inputs (numpy) and
returns the FULL output, running a Bass/Tile kernel over 8 NeuronCores
(pure data parallel over the window-batch dim).

Hardcoded problem shapes:
  x    (3136, 64, 256) f32   -> 392 windows / core
  mask (49, 64, 64) f32      (zeros in practice; general path supported)
  out  (3136, 64, 256) f32
"""

import sys

sys.path.insert(0, "/opt/trn_rl_repo")

import numpy as np
import ml_dtypes

import concourse.bass as bass
import concourse.tile as tile
from concourse import bacc, mybir
from concourse.masks import make_identity

BF16 = mybir.dt.bfloat16
F32 = mybir.dt.float32
AF = mybir.ActivationFunctionType
ALU = mybir.AluOpType

# ---- problem constants ----------------------------------------------------
DIM = 256
HEADS = 8
HD = 32
NTOK = 64
NW49 = 49
BATCH = 64
B_ = BATCH * NW49          # 3136
NCORES = 8
WPC = B_ // NCORES         # 392 windows per core
TPC = WPC * NTOK           # 25088 tokens per core
CHW = 8                    # windows per chunk
NCHUNK = WPC // CHW        # 49
SCALE = HD ** -0.5


def _np_bf16(a):
    return np.asarray(a, dtype=ml_dtypes.bfloat16)


# ---- host-side DPB MLP + relative-position gather (tiny, input-dependent) --
def _host_dpb_table(inputs, mask_nonzero):
    """Returns E table(s): exp(rpb [+ mask]) in (q, r, hh, k) layout, bf16.

    rpb[h, q, k] = p3[rel_idx[q, k], h] where p3 = DPB MLP(biases).
    Layout col index = r*... : head h = hh*4 + r  (r = PSUM bank, hh = half).
    """
    f = lambda k: np.asarray(inputs[k], np.float32)
    biases = f("biases")            # (225, 2)
    eps = 1e-5

    def ln(x, g, b):
        m = x.mean(-1, keepdims=True)
        v = ((x - m) ** 2).mean(-1, keepdims=True)
        return (x - m) / np.sqrt(v + eps) * g + b

    p = biases @ f("pos_proj_w").T + f("pos_proj_b")
    p = np.maximum(ln(p, f("ln1_g"), f("ln1_b")), 0.0) @ f("fc1_w").T + f("fc1_b")
    p = np.maximum(ln(p, f("ln2_g"), f("ln2_b")), 0.0) @ f("fc2_w").T + f("fc2_b")
    p = np.maximum(ln(p, f("ln3_g"), f("ln3_b")), 0.0) @ f("fc3_w").T + f("fc3_b")
    # p: (225, HEADS)
    rel_idx = np.asarray(inputs["rel_idx"], np.int64)      # (64, 64)
    rpb = p[rel_idx]                                        # (64, 64, HEADS) = (q, k, h)
    rpb = np.transpose(rpb, (2, 0, 1))                      # (h, q, k)
    # reorder heads to (r, hh) col layout: col block index g = r*2 + hh ->
    # we store free layout (r, hh, k): head h = hh*4 + r
    ord_heads = np.empty((4, 2, NTOK, NTOK), np.float32)
    for r in range(4):
        for hh in range(2):
            ord_heads[r, hh] = rpb[hh * 4 + r]
    # (q, r, hh, k)
    rpb_q = np.transpose(ord_heads, (2, 0, 1, 3)).reshape(NTOK, 4 * 2 * NTOK)
    if not mask_nonzero:
        e = np.exp(rpb_q)                                   # (64, 512)
        e2 = np.concatenate([e, e], axis=0)                 # (128, 512)
        return _np_bf16(e2), None
    # general path: per window-type table exp(rpb + mask[t])
    mask = np.asarray(inputs["mask"], np.float32)           # (49, 64, 64)
    ef = np.empty((NTOK, NW49, 512), np.float32)
    rq = rpb_q.reshape(NTOK, 4, 2, NTOK)
    for t in range(NW49):
        ef[:, t, :] = np.exp(rq + mask[t][:, None, None, :]).reshape(NTOK, 512)
    return None, _np_bf16(ef)


# ---- device kernel builder -------------------------------------------------
def _build(mask_nonzero, qkvb_nonzero, projb_nonzero):
    nc = bacc.Bacc("TRN2", target_bir_lowering=False, debug=False)

    x_d = nc.dram_tensor("x", (TPC, DIM), BF16, kind="ExternalInput")
    y_d = nc.dram_tensor("y", (TPC, DIM), F32, kind="ExternalOutput")
    wqk_d = nc.dram_tensor("wqk", (2, 128, 512), BF16, kind="ExternalInput")
    wv_d = nc.dram_tensor("wv", (2, 128, 256), BF16, kind="ExternalInput")
    pw_d = nc.dram_tensor("pw", (2, 128, 256), BF16, kind="ExternalInput")
    if mask_nonzero:
        e_d = nc.dram_tensor("etab", (NTOK, NW49, 512), BF16, kind="ExternalInput")
    else:
        e_d = nc.dram_tensor("etab", (128, 512), BF16, kind="ExternalInput")
    if qkvb_nonzero:
        qkb_d = nc.dram_tensor("qkb", (4, 128), F32, kind="ExternalInput")
        vb_d = nc.dram_tensor("vb", (128, 256), BF16, kind="ExternalInput")
    if projb_nonzero:
        yb_d = nc.dram_tensor("yb", (128, 256), F32, kind="ExternalInput")

    with tile.TileContext(nc) as tc:
        with (
            tc.tile_pool(name="setup", bufs=1) as setup,
            tc.tile_pool(name="xin", bufs=2) as xin,
            tc.tile_pool(name="xts", bufs=2) as xts,
            tc.tile_pool(name="qks", bufs=2) as qks,
            tc.tile_pool(name="vs", bufs=2) as vsp,
            tc.tile_pool(name="ps", bufs=2) as psp,
            tc.tile_pool(name="pts", bufs=3) as pts,
            tc.tile_pool(name="avs", bufs=2) as avs,
            tc.tile_pool(name="ys", bufs=2) as ysp,
            tc.tile_pool(name="dst", bufs=4) as dst,
            tc.tile_pool(name="pp_mm", bufs=2, space="PSUM") as pp_mm,
            tc.tile_pool(name="pp_sc", bufs=1, space="PSUM") as pp_sc,
            tc.tile_pool(name="pp_av", bufs=2, space="PSUM") as pp_av,
        ):
            # ---- one-time setup ----
            ident = setup.tile([128, 128], BF16)
            make_identity(nc, ident)

            wqk = setup.tile([128, 2, 512], BF16)
            nc.gpsimd.dma_start(
                out=wqk,
                in_=bass.AP(tensor=wqk_d, offset=0,
                            ap=[[512, 128], [128 * 512, 2], [1, 512]]),
            )
            wv = setup.tile([128, 2, 256], BF16)
            nc.gpsimd.dma_start(
                out=wv,
                in_=bass.AP(tensor=wv_d, offset=0,
                            ap=[[256, 128], [128 * 256, 2], [1, 256]]),
            )
            pw = setup.tile([128, 2, 256], BF16)
            nc.gpsimd.dma_start(
                out=pw,
                in_=bass.AP(tensor=pw_d, offset=0,
                            ap=[[256, 128], [128 * 256, 2], [1, 256]]),
            )
            if mask_nonzero:
                etab = setup.tile([NTOK, NW49, 512], BF16)
                nc.gpsimd.dma_start(out=etab, in_=e_d.ap())
            else:
                etab = setup.tile([128, 512], BF16)
                nc.gpsimd.dma_start(out=etab, in_=e_d.ap())
            if qkvb_nonzero:
                qkb = setup.tile([128, 4], F32)
                nc.gpsimd.dma_start(
                    out=qkb,
                    in_=bass.AP(tensor=qkb_d, offset=0, ap=[[1, 128], [128, 4]]),
                )
                vb = setup.tile([128, 256], BF16)
                nc.gpsimd.dma_start(out=vb, in_=vb_d.ap())
            if projb_nonzero:
                yb = setup.tile([128, 256], F32)
                nc.gpsimd.dma_start(out=yb, in_=yb_d.ap())

            # ---- main loop over chunks of 8 windows (512 tokens) ----
            for c in range(NCHUNK):
                tok0 = c * 512

                # x load (bf16, token-major): (128, 4 blk, 256)
                xb = xin.tile([128, 4, 256], BF16)
                nc.gpsimd.dma_start(
                    out=xb,
                    in_=bass.AP(tensor=x_d, offset=tok0 * DIM,
                                ap=[[DIM, 128], [128 * DIM, 4], [1, DIM]]),
                )

                # transpose x -> xT (c-major): psum (128, 2 kk, 512 tok) bf16
                xtp = pp_mm.tile([128, 2, 512], BF16, name="xtp", tag="mm")
                for blk in range(4):
                    for kk in range(2):
                        nc.tensor.transpose(
                            xtp[:, kk, 128 * blk:128 * (blk + 1)],
                            xb[:, blk, 128 * kk:128 * (kk + 1)],
                            ident,
                        )
                xt = xts.tile([128, 2, 512], BF16)
                nc.vector.tensor_copy(xt, xtp)

                # qkT = Wqk.T @ xT : 4 m-tiles (q0 q1 k0 k1), psum (128,512)
                qk = qks.tile([128, 4, 512], BF16)
                for m in range(4):
                    qkp = pp_mm.tile([128, 512], F32, name="qkp", tag="mm")
                    for kk in range(2):
                        nc.tensor.matmul(
                            qkp,
                            wqk[:, kk, 128 * m:128 * (m + 1)],
                            xt[:, kk, :],
                            start=(kk == 0),
                            stop=(kk == 1),
                        )
                    if qkvb_nonzero:
                        if m < 2:
                            nc.scalar.activation(
                                qk[:, m, :], qkp, AF.Copy, bias=qkb[:, m:m + 1]
                            )
                        else:
                            nc.vector.tensor_scalar_add(
                                qk[:, m, :], qkp, qkb[:, m:m + 1]
                            )
                    else:
                        if m < 2:
                            nc.scalar.copy(qk[:, m, :], qkp)
                        else:
                            nc.vector.tensor_copy(qk[:, m, :], qkp)

                # v (token-major) per pair j: psum (128, 256)
                v_sb = vsp.tile([128, 4, 256], BF16)
                vd_sb = vsp.tile([128, 4, 256], BF16)
                for j in range(4):
                    vp = pp_mm.tile([128, 256], F32, name="vp", tag="mm")
                    for kk in range(2):
                        nc.tensor.matmul(
                            vp,
                            xt[:, kk, 128 * j:128 * (j + 1)],
                            wv[:, kk, :],
                            start=(kk == 0),
                            stop=(kk == 1),
                        )
                    if qkvb_nonzero:
                        nc.vector.tensor_tensor(v_sb[:, j, :], vp, vb, ALU.add)
                    else:
                        nc.scalar.copy(v_sb[:, j, :], vp)
                    # duplicate with swapped 64-row halves (for head-parity)
                    nc.gpsimd.dma_start(out=vd_sb[0:64, j, :], in_=v_sb[64:128, j, :])
                    nc.gpsimd.dma_start(out=vd_sb[64:128, j, :], in_=v_sb[0:64, j, :])

                # scores: psum (128, 2048) f32 = 4 banks, layout (r, j, hh, k)
                sc = pp_sc.tile([128, 2048], F32)
                scv = sc.rearrange("p (r j hh k) -> p r j hh k", r=4, j=4, hh=2)
                for j in range(4):
                    for hh in range(2):
                        for win in range(2):
                            for r in range(4):
                                nc.tensor.matmul(
                                    scv[64 * win:64 * (win + 1), r, j, hh, :],
                                    qk[32 * r:32 * r + 32, hh,
                                       128 * j + 64 * win:128 * j + 64 * win + 64],
                                    qk[32 * r:32 * r + 32, 2 + hh,
                                       128 * j + 64 * win:128 * j + 64 * win + 64],
                                    tile_position=(32 * r, 64 * win),
                                )

                # exp (ACT): psum f32 -> sbuf bf16
                p_sb = psp.tile([128, 2048], BF16)
                nc.scalar.activation(p_sb, sc, AF.Exp)

                # multiply by exp(rpb [+ mask]) table
                pv = p_sb.rearrange("p (r j hh k) -> p r j hh k", r=4, j=4, hh=2)
                if not mask_nonzero:
                    e_ap = bass.AP(
                        tensor=etab.tensor,
                        offset=etab.offset,
                        ap=[etab.ap[0], [128, 4], [0, 4], [64, 2], [1, 64]],
                    )
                    nc.vector.tensor_tensor(pv, pv, e_ap, ALU.mult)
                else:
                    for w in range(CHW):
                        t49 = (c * CHW + w) % NW49
                        j, win = w // 2, w % 2
                        dst_ap = pv[64 * win:64 * (win + 1), :, j, :, :]
                        e_ap = bass.AP(
                            tensor=etab.tensor,
                            offset=etab.offset + t49 * 512,
                            ap=[etab.ap[0], [128, 4], [64, 2], [1, 64]],
                        )
                        nc.vector.tensor_tensor(dst_ap, dst_ap, e_ap, ALU.mult)

                # denominators + reciprocal + normalize
                dsum = dst.tile([128, 32], F32)
                nc.vector.tensor_reduce(
                    dsum, p_sb.rearrange("p (g k) -> p g k", g=32),
                    axis=mybir.AxisListType.X, op=ALU.add,
                )
                rec = dst.tile([128, 32], F32)
                nc.vector.reciprocal_approx_fast(rec, dsum)
                recb = dst.tile([128, 32], BF16)
                nc.vector.tensor_copy(recb, rec)
                r_ap = bass.AP(
                    tensor=recb.tensor, offset=recb.offset,
                    ap=[recb.ap[0], [1, 32], [0, 64]],
                )
                p3v = p_sb.rearrange("p (g k) -> p g k", g=32)
                nc.vector.tensor_tensor(p3v, p3v, r_ap, ALU.mult)

                # per pair: transpose probs, evac, av matmuls
                avp = [
                    pp_av.tile([128, 512], F32, name=f"avp{hh}", tag="avp")
                    for hh in range(2)
                ]
                for j in range(4):
                    ptp = pp_mm.tile([128, 512], BF16, name="ptp", tag="mm")
                    for r in range(4):
                        nc.tensor.transpose(
                            ptp[:, 128 * r:128 * (r + 1)],
                            p_sb[:, 512 * r + 128 * j:512 * r + 128 * j + 128],
                            ident,
                        )
                    pt_sb = pts.tile([128, 512], BF16)
                    nc.vector.tensor_copy(pt_sb, ptp)
                    for hh in range(2):
                        for win in range(2):
                            vt = v_sb if win == hh else vd_sb
                            for r in range(4):
                                h = hh * 4 + r
                                nc.tensor.matmul(
                                    avp[hh][32 * r:32 * r + 32,
                                            64 * (2 * j + win):64 * (2 * j + win) + 64],
                                    vt[64 * hh:64 * hh + 64, j,
                                       32 * h:32 * h + 32],
                                    pt_sb[64 * hh:64 * hh + 64,
                                          128 * r + 64 * win:128 * r + 64 * win + 64],
                                    tile_position=(64 * hh, 32 * r),
                                )

                # evac av (feat-major out) then proj
                av_sb = avs.tile([128, 2, 512], BF16)
                nc.scalar.copy(av_sb[:, 0, :], avp[0])
                nc.vector.tensor_copy(av_sb[:, 1, :], avp[1])

                y_sb = ysp.tile([128, 4, 256], F32)
                for j in range(4):
                    yp = pp_mm.tile([128, 256], F32, name="yp", tag="mm")
                    for kk in range(2):
                        nc.tensor.matmul(
                            yp,
                            av_sb[:, kk, 128 * j:128 * (j + 1)],
                            pw[:, kk, :],
                            start=(kk == 0),
                            stop=(kk == 1),
                        )
                    if projb_nonzero:
                        nc.vector.tensor_tensor(y_sb[:, j, :], yp, yb, ALU.add)
                    else:
                        if j < 2:
                            nc.vector.tensor_copy(y_sb[:, j, :], yp)
                        else:
                            nc.scalar.copy(y_sb[:, j, :], yp)

                nc.gpsimd.dma_start(
                    out=bass.AP(tensor=y_d, offset=tok0 * DIM,
                                ap=[[DIM, 128], [128 * DIM, 4], [1, DIM]]),
                    in_=y_sb,
                )

    nc.compile()
    return nc


# ---- execution --------------------------------------------------------------
_CACHE = {}


def _get_runner(mask_nonzero, qkvb_nonzero, projb_nonzero):
    key = (mask_nonzero, qkvb_nonzero, projb_nonzero)
    if key in _CACHE:
        return _CACHE[key]

    nc = _build(mask_nonzero, qkvb_nonzero, projb_nonzero)

    import jax
    import jax.numpy as jnp
    from jax.sharding import Mesh, PartitionSpec
    from jax.experimental.shard_map import shard_map
    from concourse import bass2jax
    from concourse.bass2jax import _bass_exec_p, install_neuronx_cc_hook

    install_neuronx_cc_hook()

    partition_name = (
        nc.partition_id_tensor.name if nc.partition_id_tensor else None
    )
    in_names, out_names, out_avals, zero_outs = [], [], [], []
    for alloc in nc.m.functions[0].allocations:
        if not isinstance(alloc, mybir.MemoryLocationSet):
            continue
        name = alloc.memorylocations[0].name
        if alloc.kind == "ExternalInput":
            if name != partition_name:
                in_names.append(name)
        elif alloc.kind == "ExternalOutput":
            shape = tuple(alloc.tensor_shape)
            dtype = mybir.dt.np(alloc.dtype)
            out_names.append(name)
            out_avals.append(jax.core.ShapedArray(shape, dtype))
            zero_outs.append(np.zeros(shape, dtype))
    n_params = len(in_names)
    n_outs = len(out_avals)
    all_in_names = list(in_names) + list(out_names)
    if partition_name is not None:
        all_in_names.append(partition_name)

    def _body(*args):
        operands = list(args)
        if partition_name is not None:
            operands.append(bass2jax.partition_id_tensor())
        outs = _bass_exec_p.bind(
            *operands,
            out_avals=tuple(out_avals),
            in_names=tuple(all_in_names),
            out_names=tuple(out_names),
            lowering_input_output_aliases=(),
            sim_require_finite=True,
            sim_require_nnan=True,
            nc=nc,
        )
        return tuple(outs)

    devices = jax.devices()[:NCORES]
    mesh = Mesh(np.asarray(devices), ("core",))
    donate = tuple(range(n_params, n_params + n_outs))
    sharded = jax.jit(
        shard_map(
            _body, mesh=mesh,
            in_specs=(PartitionSpec("core"),) * (n_params + n_outs),
            out_specs=(PartitionSpec("core"),) * n_outs,
            check_rep=False,
        ),
        donate_argnums=donate,
        keep_unused=True,
    )

    def run(in_maps):
        concat_in = [
            np.concatenate([np.asarray(in_maps[c][nm]) for c in range(NCORES)], axis=0)
            for nm in in_names
        ]
        concat_zero = [
            np.zeros((NCORES * z.shape[0], *z.shape[1:]), z.dtype) for z in zero_outs
        ]
        out_arrs = sharded(*concat_in, *concat_zero)
        out = np.asarray(out_arrs[out_names.index("y")])
        return out.reshape(NCORES, TPC, DIM)

    _CACHE[key] = run
    return run


def _prep_in_maps(inputs):
    x = np.asarray(inputs["x"], np.float32).reshape(B_ * NTOK, DIM)
    qkv_w = np.asarray(inputs["qkv_w"], np.float32)
    qkv_b = np.asarray(inputs["qkv_b"], np.float32)
    proj_w = np.asarray(inputs["proj_w"], np.float32)
    proj_b = np.asarray(inputs["proj_b"], np.float32)
    mask_nonzero = bool(np.any(np.asarray(inputs["mask"]) != 0))
    qkvb_nonzero = bool(np.any(qkv_b != 0))
    projb_nonzero = bool(np.any(proj_b != 0))

    wqk_f = qkv_w[:512].copy()
    wqk_f[:256] *= SCALE                       # fold q scale into Wq
    wqk = _np_bf16(wqk_f.T.reshape(2, 128, 512))
    wv = _np_bf16(qkv_w[512:].T.reshape(2, 128, 256))
    pw = _np_bf16(proj_w.T.reshape(2, 128, 256))

    e2, efull = _host_dpb_table(inputs, mask_nonzero)
    etab = efull if mask_nonzero else e2

    shared = {"wqk": wqk, "wv": wv, "pw": pw, "etab": etab}
    if qkvb_nonzero:
        qkb_f = qkv_b[:512].copy()
        qkb_f[:256] *= SCALE
        shared["qkb"] = qkb_f.reshape(4, 128).astype(np.float32)
        shared["vb"] = _np_bf16(np.broadcast_to(qkv_b[512:], (128, 256)).copy())
    if projb_nonzero:
        shared["yb"] = np.broadcast_to(proj_b, (128, 256)).copy().astype(np.float32)

    xb = _np_bf16(x)
    in_maps = []
    for c in range(NCORES):
        m = dict(shared)
        m["x"] = xb[c * TPC:(c + 1) * TPC]
        in_maps.append(m)
    flags = (mask_nonzero, qkvb_nonzero, projb_nonzero)
    return in_maps, flags


def kernel(**inputs) -> np.ndarray:
    in_maps, flags = _prep_in_maps(inputs)
    run = _get_runner(*flags)
    out = run(in_maps)                          # (8, TPC, DIM) f32
    return out.reshape(B_, NTOK, DIM)


# revision 6
# speedup vs baseline: 81.6570x; 81.6570x over previous
"""DPB (dynamic position bias) window attention kernel for Trainium2.

Contract: kernel(**inputs) takes the FULL unsharded inputs (numpy) and
returns the FULL output, running a Bass/Tile kernel over 8 NeuronCores
(pure data parallel over the window-batch dim).

Hardcoded problem shapes:
  x    (3136, 64, 256) f32   -> 392 windows / core
  mask (49, 64, 64) f32      (zeros in practice; general path supported)
  out  (3136, 64, 256) f32
"""

import sys

sys.path.insert(0, "/opt/trn_rl_repo")

import numpy as np
import ml_dtypes

import concourse.bass as bass
import concourse.tile as tile
from concourse import bacc, mybir
from concourse.masks import make_identity

BF16 = mybir.dt.bfloat16
F32 = mybir.dt.float32
AF = mybir.ActivationFunctionType
ALU = mybir.AluOpType

# ---- problem constants ----------------------------------------------------
DIM = 256
HEADS = 8
HD = 32
NTOK = 64
NW49 = 49
BATCH = 64
B_ = BATCH * NW49          # 3136
NCORES = 8
WPC = B_ // NCORES         # 392 windows per core
TPC = WPC * NTOK           # 25088 tokens per core
CHW = 8                    # windows per chunk
NCHUNK = WPC // CHW        # 49
SCALE = HD ** -0.5


def _np_bf16(a):
    return np.asarray(a, dtype=ml_dtypes.bfloat16)


# ---- host-side DPB MLP + relative-position gather (tiny, input-dependent) --
def _host_dpb_table(inputs, mask_nonzero):
    """Returns E table(s): exp(rpb [+ mask]) in (q, r, hh, k) layout, bf16.

    rpb[h, q, k] = p3[rel_idx[q, k], h] where p3 = DPB MLP(biases).
    Layout col index = r*... : head h = hh*4 + r  (r = PSUM bank, hh = half).
    """
    f = lambda k: np.asarray(inputs[k], np.float32)
    biases = f("biases")            # (225, 2)
    eps = 1e-5

    def ln(x, g, b):
        m = x.mean(-1, keepdims=True)
        v = ((x - m) ** 2).mean(-1, keepdims=True)
        return (x - m) / np.sqrt(v + eps) * g + b

    p = biases @ f("pos_proj_w").T + f("pos_proj_b")
    p = np.maximum(ln(p, f("ln1_g"), f("ln1_b")), 0.0) @ f("fc1_w").T + f("fc1_b")
    p = np.maximum(ln(p, f("ln2_g"), f("ln2_b")), 0.0) @ f("fc2_w").T + f("fc2_b")
    p = np.maximum(ln(p, f("ln3_g"), f("ln3_b")), 0.0) @ f("fc3_w").T + f("fc3_b")
    # p: (225, HEADS)
    rel_idx = np.asarray(inputs["rel_idx"], np.int64)      # (64, 64)
    rpb = p[rel_idx]                                        # (64, 64, HEADS) = (q, k, h)
    rpb = np.transpose(rpb, (2, 0, 1))                      # (h, q, k)
    # reorder heads to (r, hh) col layout: col block index g = r*2 + hh ->
    # we store free layout (r, hh, k): head h = hh*4 + r
    ord_heads = np.empty((4, 2, NTOK, NTOK), np.float32)
    for r in range(4):
        for hh in range(2):
            ord_heads[r, hh] = rpb[hh * 4 + r]
    # (q, r, hh, k)
    rpb_q = np.transpose(ord_heads, (2, 0, 1, 3)).reshape(NTOK, 4 * 2 * NTOK)
    if not mask_nonzero:
        e = np.exp(rpb_q)                                   # (64, 512)
        e2 = np.concatenate([e, e], axis=0)                 # (128, 512)
        return _np_bf16(e2), None
    # general path: per window-type table exp(rpb + mask[t])
    mask = np.asarray(inputs["mask"], np.float32)           # (49, 64, 64)
    ef = np.empty((NTOK, NW49, 512), np.float32)
    rq = rpb_q.reshape(NTOK, 4, 2, NTOK)
    for t in range(NW49):
        ef[:, t, :] = np.exp(rq + mask[t][:, None, None, :]).reshape(NTOK, 512)
    return None, _np_bf16(ef)


# ---- device kernel builder -------------------------------------------------
def _build(mask_nonzero, qkvb_nonzero, projb_nonzero):
    nc = bacc.Bacc("TRN2", target_bir_lowering=False, debug=False)

    x_d = nc.dram_tensor("x", (TPC, DIM), BF16, kind="ExternalInput")
    y_d = nc.dram_tensor("y", (TPC, DIM), F32, kind="ExternalOutput")
    wqk_d = nc.dram_tensor("wqk", (2, 128, 512), BF16, kind="ExternalInput")
    wv_d = nc.dram_tensor("wv", (2, 128, 256), BF16, kind="ExternalInput")
    pw_d = nc.dram_tensor("pw", (2, 128, 256), BF16, kind="ExternalInput")
    if mask_nonzero:
        e_d = nc.dram_tensor("etab", (NTOK, NW49, 512), BF16, kind="ExternalInput")
    else:
        e_d = nc.dram_tensor("etab", (128, 512), BF16, kind="ExternalInput")
    if qkvb_nonzero:
        qkb_d = nc.dram_tensor("qkb", (4, 128), F32, kind="ExternalInput")
        vb_d = nc.dram_tensor("vb", (128, 256), BF16, kind="ExternalInput")
    if projb_nonzero:
        yb_d = nc.dram_tensor("yb", (128, 256), F32, kind="ExternalInput")

    with tile.TileContext(nc) as tc:
        with (
            tc.tile_pool(name="setup", bufs=1) as setup,
            tc.tile_pool(name="xin", bufs=2) as xin,
            tc.tile_pool(name="xts", bufs=2) as xts,
            tc.tile_pool(name="qks", bufs=2) as qks,
            tc.tile_pool(name="vs", bufs=2) as vsp,
            tc.tile_pool(name="ps", bufs=2) as psp,
            tc.tile_pool(name="pts", bufs=3) as pts,
            tc.tile_pool(name="avs", bufs=2) as avs,
            tc.tile_pool(name="ys", bufs=2) as ysp,
            tc.tile_pool(name="dst", bufs=4) as dst,
            tc.tile_pool(name="pp_mm", bufs=2, space="PSUM") as pp_mm,
            tc.tile_pool(name="pp_sc", bufs=1, space="PSUM") as pp_sc,
            tc.tile_pool(name="pp_av", bufs=2, space="PSUM") as pp_av,
        ):
            # ---- one-time setup ----
            ident = setup.tile([128, 128], BF16)
            make_identity(nc, ident)

            wqk = setup.tile([128, 2, 512], BF16)
            nc.gpsimd.dma_start(
                out=wqk,
                in_=bass.AP(tensor=wqk_d, offset=0,
                            ap=[[512, 128], [128 * 512, 2], [1, 512]]),
            )
            wv = setup.tile([128, 2, 256], BF16)
            nc.gpsimd.dma_start(
                out=wv,
                in_=bass.AP(tensor=wv_d, offset=0,
                            ap=[[256, 128], [128 * 256, 2], [1, 256]]),
            )
            pw = setup.tile([128, 2, 256], BF16)
            nc.gpsimd.dma_start(
                out=pw,
                in_=bass.AP(tensor=pw_d, offset=0,
                            ap=[[256, 128], [128 * 256, 2], [1, 256]]),
            )
            if mask_nonzero:
                etab = setup.tile([NTOK, NW49, 512], BF16)
                nc.gpsimd.dma_start(out=etab, in_=e_d.ap())
            else:
                etab = setup.tile([128, 512], BF16)
                nc.gpsimd.dma_start(out=etab, in_=e_d.ap())
            if qkvb_nonzero:
                qkb = setup.tile([128, 4], F32)
                nc.gpsimd.dma_start(
                    out=qkb,
                    in_=bass.AP(tensor=qkb_d, offset=0, ap=[[1, 128], [128, 4]]),
                )
                vb = setup.tile([128, 256], BF16)
                nc.gpsimd.dma_start(out=vb, in_=vb_d.ap())
            if projb_nonzero:
                yb = setup.tile([128, 256], F32)
                nc.gpsimd.dma_start(out=yb, in_=yb_d.ap())

            # ---- main loop over chunks of 8 windows (512 tokens) ----
            for c in range(NCHUNK):
                tok0 = c * 512

                # x load (bf16, token-major): (128, 4 blk, 256)
                xb = xin.tile([128, 4, 256], BF16)
                nc.gpsimd.dma_start(
                    out=xb,
                    in_=bass.AP(tensor=x_d, offset=tok0 * DIM,
                                ap=[[DIM, 128], [128 * DIM, 4], [1, DIM]]),
                )

                # transpose x -> xT (c-major): psum (128, 2 kk, 512 tok) bf16
                xtp = pp_mm.tile([128, 2, 512], BF16, name="xtp", tag="mm")
                for blk in range(4):
                    for kk in range(2):
                        nc.tensor.transpose(
                            xtp[:, kk, 128 * blk:128 * (blk + 1)],
                            xb[:, blk, 128 * kk:128 * (kk + 1)],
                            ident,
                        )
                xt = xts.tile([128, 2, 512], BF16)
                nc.vector.tensor_copy(xt, xtp)

                # qkT = Wqk.T @ xT : 4 m-tiles (q0 q1 k0 k1), psum (128,512)
                qk = qks.tile([128, 4, 512], BF16)
                for m in range(4):
                    qkp = pp_mm.tile([128, 512], F32, name="qkp", tag="mm")
                    for kk in range(2):
                        nc.tensor.matmul(
                            qkp,
                            wqk[:, kk, 128 * m:128 * (m + 1)],
                            xt[:, kk, :],
                            start=(kk == 0),
                            stop=(kk == 1),
                        )
                    if qkvb_nonzero:
                        if m < 2:
                            nc.scalar.activation(
                                qk[:, m, :], qkp, AF.Copy, bias=qkb[:, m:m + 1]
                            )
                        else:
                            nc.vector.tensor_scalar_add(
                                qk[:, m, :], qkp, qkb[:, m:m + 1]
                            )
                    else:
                        if m < 2:
                            nc.scalar.copy(qk[:, m, :], qkp)
                        else:
                            nc.vector.tensor_copy(qk[:, m, :], qkp)

                # v (token-major) per pair j: psum (128, 256)
                v_sb = vsp.tile([128, 4, 256], BF16)
                vd_sb = vsp.tile([128, 4, 256], BF16)
                for j in range(4):
                    vp = pp_mm.tile([128, 256], F32, name="vp", tag="mm")
                    for kk in range(2):
                        nc.tensor.matmul(
                            vp,
                            xt[:, kk, 128 * j:128 * (j + 1)],
                            wv[:, kk, :],
                            start=(kk == 0),
                            stop=(kk == 1),
                        )
                    if qkvb_nonzero:
                        nc.vector.tensor_tensor(v_sb[:, j, :], vp, vb, ALU.add)
                    else:
                        nc.scalar.copy(v_sb[:, j, :], vp)
                    # duplicate with swapped 64-row halves (for head-parity)
                    nc.gpsimd.dma_start(out=vd_sb[0:64, j, :], in_=v_sb[64:128, j, :])
                    nc.gpsimd.dma_start(out=vd_sb[64:128, j, :], in_=v_sb[0:64, j, :])

                # scores: psum (128, 2048) f32 = 4 banks, layout (r, j, hh, k)
                sc = pp_sc.tile([128, 2048], F32)
                scv = sc.rearrange("p (r j hh k) -> p r j hh k", r=4, j=4, hh=2)
                for j in range(4):
                    for hh in range(2):
                        for win in range(2):
                            for r in range(4):
                                nc.tensor.matmul(
                                    scv[64 * win:64 * (win + 1), r, j, hh, :],
                                    qk[32 * r:32 * r + 32, hh,
                                       128 * j + 64 * win:128 * j + 64 * win + 64],
                                    qk[32 * r:32 * r + 32, 2 + hh,
                                       128 * j + 64 * win:128 * j + 64 * win + 64],
                                    tile_position=(32 * r, 64 * win),
                                )

                # exp (ACT): psum f32 -> sbuf bf16
                p_sb = psp.tile([128, 2048], BF16)
                nc.scalar.activation(p_sb, sc, AF.Exp)

                # multiply by exp(rpb [+ mask]) table
                pv = p_sb.rearrange("p (r j hh k) -> p r j hh k", r=4, j=4, hh=2)
                if not mask_nonzero:
                    e_ap = bass.AP(
                        tensor=etab.tensor,
                        offset=etab.offset,
                        ap=[etab.ap[0], [128, 4], [0, 4], [64, 2], [1, 64]],
                    )
                    nc.vector.tensor_tensor(pv, pv, e_ap, ALU.mult)
                else:
                    for w in range(CHW):
                        t49 = (c * CHW + w) % NW49
                        j, win = w // 2, w % 2
                        dst_ap = pv[64 * win:64 * (win + 1), :, j, :, :]
                        e_ap = bass.AP(
                            tensor=etab.tensor,
                            offset=etab.offset + t49 * 512,
                            ap=[etab.ap[0], [128, 4], [64, 2], [1, 64]],
                        )
                        nc.vector.tensor_tensor(dst_ap, dst_ap, e_ap, ALU.mult)

                # denominators + reciprocal + normalize
                dsum = dst.tile([128, 32], F32)
                nc.vector.tensor_reduce(
                    dsum, p_sb.rearrange("p (g k) -> p g k", g=32),
                    axis=mybir.AxisListType.X, op=ALU.add,
                )
                rec = dst.tile([128, 32], F32)
                nc.vector.reciprocal_approx_fast(rec, dsum)
                recb = dst.tile([128, 32], BF16)
                nc.vector.tensor_copy(recb, rec)
                r_ap = bass.AP(
                    tensor=recb.tensor, offset=recb.offset,
                    ap=[recb.ap[0], [1, 32], [0, 64]],
                )
                p3v = p_sb.rearrange("p (g k) -> p g k", g=32)
                nc.vector.tensor_tensor(p3v, p3v, r_ap, ALU.mult)

                # per pair: transpose probs, evac, av matmuls
                avp = [
                    pp_av.tile([128, 512], F32, name=f"avp{hh}", tag="avp")
                    for hh in range(2)
                ]
                for j in range(4):
                    ptp = pp_mm.tile([128, 512], BF16, name="ptp", tag="mm")
                    for r in range(4):
                        nc.tensor.transpose(
                            ptp[:, 128 * r:128 * (r + 1)],
                            p_sb[:, 512 * r + 128 * j:512 * r + 128 * j + 128],
                            ident,
                        )
                    pt_sb = pts.tile([128, 512], BF16)
                    nc.vector.tensor_copy(pt_sb, ptp)
                    for hh in range(2):
                        for win in range(2):
                            vt = v_sb if win == hh else vd_sb
                            for r in range(4):
                                h = hh * 4 + r
                                nc.tensor.matmul(
                                    avp[hh][32 * r:32 * r + 32,
                                            64 * (2 * j + win):64 * (2 * j + win) + 64],
                                    vt[64 * hh:64 * hh + 64, j,
                                       32 * h:32 * h + 32],
                                    pt_sb[64 * hh:64 * hh + 64,
                                          128 * r + 64 * win:128 * r + 64 * win + 64],
                                    tile_position=(64 * hh, 32 * r),
                                )

                # evac av (feat-major out) then proj
                av_sb = avs.tile([128, 2, 512], BF16)
                nc.scalar.copy(av_sb[:, 0, :], avp[0])
                nc.vector.tensor_copy(av_sb[:, 1, :], avp[1])

                y_sb = ysp.tile([128, 4, 256], F32)
                for j in range(4):
                    yp = pp_mm.tile([128, 256], F32, name="yp", tag="mm")
                    for kk in range(2):
                        nc.tensor.matmul(
                            yp,
                            av_sb[:, kk, 128 * j:128 * (j + 1)],
                            pw[:, kk, :],
                            start=(kk == 0),
                            stop=(kk == 1),
                        )
                    if projb_nonzero:
                        nc.vector.tensor_tensor(y_sb[:, j, :], yp, yb, ALU.add)
                    else:
                        if j < 2:
                            nc.vector.tensor_copy(y_sb[:, j, :], yp)
                        else:
                            nc.scalar.copy(y_sb[:, j, :], yp)

                nc.gpsimd.dma_start(
                    out=bass.AP(tensor=y_d, offset=tok0 * DIM,
                                ap=[[DIM, 128], [128 * DIM, 4], [1, DIM]]),
                    in_=y_sb,
                )

    nc.compile()
    return nc


# ---- execution --------------------------------------------------------------
_CACHE = {}


def _get_runner(mask_nonzero, qkvb_nonzero, projb_nonzero):
    key = (mask_nonzero, qkvb_nonzero, projb_nonzero)
    if key in _CACHE:
        return _CACHE[key]

    nc = _build(mask_nonzero, qkvb_nonzero, projb_nonzero)

    import jax
    import jax.numpy as jnp
    from jax.sharding import Mesh, PartitionSpec
    from jax.experimental.shard_map import shard_map
    from concourse import bass2jax
    from concourse.bass2jax import _bass_exec_p, install_neuronx_cc_hook

    install_neuronx_cc_hook()

    partition_name = (
        nc.partition_id_tensor.name if nc.partition_id_tensor else None
    )
    in_names, out_names, out_avals, zero_outs = [], [], [], []
    for alloc in nc.m.functions[0].allocations:
        if not isinstance(alloc, mybir.MemoryLocationSet):
            continue
        name = alloc.memorylocations[0].name
        if alloc.kind == "ExternalInput":
            if name != partition_name:
                in_names.append(name)
        elif alloc.kind == "ExternalOutput":
            shape = tuple(alloc.tensor_shape)
            dtype = mybir.dt.np(alloc.dtype)
            out_names.append(name)
            out_avals.append(jax.core.ShapedArray(shape, dtype))
            zero_outs.append(np.zeros(shape, dtype))
    n_params = len(in_names)
    n_outs = len(out_avals)
    all_in_names = list(in_names) + list(out_names)
    if partition_name is not None:
        all_in_names.append(partition_name)

    def _body(*args):
        operands = list(args)
        if partition_name is not None:
            operands.append(bass2jax.partition_id_tensor())
        outs = _bass_exec_p.bind(
            *operands,
            out_avals=tuple(out_avals),
            in_names=tuple(all_in_names),
            out_names=tuple(out_names),
            lowering_input_output_aliases=(),
            sim_require_finite=True,
            sim_require_nnan=True,
            nc=nc,
        )
        return tuple(outs)

    devices = jax.devices()[:NCORES]
    mesh = Mesh(np.asarray(devices), ("core",))
    donate = tuple(range(n_params, n_params + n_outs))
    sharded = jax.jit(
        shard_map(
            _body, mesh=mesh,
            in_specs=(PartitionSpec("core"),) * (n_params + n_outs),
            out_specs=(PartitionSpec("core"),) * n_outs,
            check_rep=False,
        ),
        donate_argnums=donate,
        keep_unused=True,
    )

    from jax.sharding import NamedSharding

    shard = NamedSharding(mesh, PartitionSpec("core"))
    zero_shapes = [
        ((NCORES * z.shape[0], *z.shape[1:]), z.dtype) for z in zero_outs
    ]
    make_zeros = jax.jit(
        lambda: tuple(jnp.zeros(s, d) for s, d in zero_shapes),
        out_shardings=(shard,) * n_outs,
    )

    def _concat(in_maps):
        return [
            np.concatenate([np.asarray(in_maps[c][nm]) for c in range(NCORES)], axis=0)
            for nm in in_names
        ]

    def run(in_maps):
        out_arrs = sharded(*_concat(in_maps), *make_zeros())
        out = np.asarray(out_arrs[out_names.index("y")])
        return out.reshape(NCORES, TPC, DIM)

    def bench(in_maps, iters=8):
        import time as _time

        dev_in = [jax.device_put(a, shard) for a in _concat(in_maps)]
        jax.block_until_ready(dev_in)
        outs = sharded(*dev_in, *make_zeros())
        jax.block_until_ready(outs)
        ts = []
        for _ in range(iters):
            t0 = _time.perf_counter()
            outs = sharded(*dev_in, *make_zeros())
            jax.block_until_ready(outs)
            ts.append(_time.perf_counter() - t0)
        return min(ts), ts

    run.bench = bench
    _CACHE[key] = run
    return run


def _prep_in_maps(inputs):
    x = np.asarray(inputs["x"], np.float32).reshape(B_ * NTOK, DIM)
    qkv_w = np.asarray(inputs["qkv_w"], np.float32)
    qkv_b = np.asarray(inputs["qkv_b"], np.float32)
    proj_w = np.asarray(inputs["proj_w"], np.float32)
    proj_b = np.asarray(inputs["proj_b"], np.float32)
    mask_nonzero = bool(np.any(np.asarray(inputs["mask"]) != 0))
    qkvb_nonzero = bool(np.any(qkv_b != 0))
    projb_nonzero = bool(np.any(proj_b != 0))

    wqk_f = qkv_w[:512].copy()
    wqk_f[:256] *= SCALE                       # fold q scale into Wq
    wqk = _np_bf16(wqk_f.T.reshape(2, 128, 512))
    wv = _np_bf16(qkv_w[512:].T.reshape(2, 128, 256))
    pw = _np_bf16(proj_w.T.reshape(2, 128, 256))

    e2, efull = _host_dpb_table(inputs, mask_nonzero)
    etab = efull if mask_nonzero else e2

    shared = {"wqk": wqk, "wv": wv, "pw": pw, "etab": etab}
    if qkvb_nonzero:
        qkb_f = qkv_b[:512].copy()
        qkb_f[:256] *= SCALE
        shared["qkb"] = qkb_f.reshape(4, 128).astype(np.float32)
        shared["vb"] = _np_bf16(np.broadcast_to(qkv_b[512:], (128, 256)).copy())
    if projb_nonzero:
        shared["yb"] = np.broadcast_to(proj_b, (128, 256)).copy().astype(np.float32)

    xb = _np_bf16(x)
    in_maps = []
    for c in range(NCORES):
        m = dict(shared)
        m["x"] = xb[c * TPC:(c + 1) * TPC]
        in_maps.append(m)
    flags = (mask_nonzero, qkvb_nonzero, projb_nonzero)
    return in_maps, flags


def kernel(**inputs) -> np.ndarray:
    in_maps, flags = _prep_in_maps(inputs)
    run = _get_runner(*flags)
    out = run(in_maps)                          # (8, TPC, DIM) f32
    return out.reshape(B_, NTOK, DIM)


# revision 22
# speedup vs baseline: 123.9465x; 1.5179x over previous
"""DPB (dynamic position bias) window attention kernel for Trainium2.

Contract: kernel(**inputs) takes the FULL unsharded inputs (numpy) and
returns the FULL output, running a Bass/Tile kernel over 8 NeuronCores
(pure data parallel over the window-batch dim).

Hardcoded problem shapes:
  x    (3136, 64, 256) f32   -> 392 windows / core
  mask (49, 64, 64) f32      (zeros in practice; general path supported)
  out  (3136, 64, 256) f32
"""

import sys

sys.path.insert(0, "/opt/trn_rl_repo")

import numpy as np
import ml_dtypes

import concourse.bass as bass
import concourse.tile as tile
from concourse import bacc, mybir
from concourse.masks import make_identity

BF16 = mybir.dt.bfloat16
F32 = mybir.dt.float32
AF = mybir.ActivationFunctionType
ALU = mybir.AluOpType

# ---- problem constants ----------------------------------------------------
DIM = 256
HEADS = 8
HD = 32
NTOK = 64
NW49 = 49
BATCH = 64
B_ = BATCH * NW49          # 3136
NCORES = 8
WPC = B_ // NCORES         # 392 windows per core
TPC = WPC * NTOK           # 25088 tokens per core
CHW = 8                    # windows per chunk
NCHUNK = WPC // CHW        # 49
SCALE = HD ** -0.5


def _np_bf16(a):
    return np.asarray(a, dtype=ml_dtypes.bfloat16)


# ---- host-side DPB MLP + relative-position gather (tiny, input-dependent) --
def _host_dpb_table(inputs, mask_nonzero):
    """Returns E table(s): exp(rpb [+ mask]) in (q, r, hh, k) layout, bf16.

    rpb[h, q, k] = p3[rel_idx[q, k], h] where p3 = DPB MLP(biases).
    Layout col index = r*... : head h = hh*4 + r  (r = PSUM bank, hh = half).
    """
    f = lambda k: np.asarray(inputs[k], np.float32)
    biases = f("biases")            # (225, 2)
    eps = 1e-5

    def ln(x, g, b):
        m = x.mean(-1, keepdims=True)
        v = ((x - m) ** 2).mean(-1, keepdims=True)
        return (x - m) / np.sqrt(v + eps) * g + b

    p = biases @ f("pos_proj_w").T + f("pos_proj_b")
    p = np.maximum(ln(p, f("ln1_g"), f("ln1_b")), 0.0) @ f("fc1_w").T + f("fc1_b")
    p = np.maximum(ln(p, f("ln2_g"), f("ln2_b")), 0.0) @ f("fc2_w").T + f("fc2_b")
    p = np.maximum(ln(p, f("ln3_g"), f("ln3_b")), 0.0) @ f("fc3_w").T + f("fc3_b")
    # p: (225, HEADS)
    rel_idx = np.asarray(inputs["rel_idx"], np.int64)      # (64, 64)
    rpb = p[rel_idx]                                        # (64, 64, HEADS) = (q, k, h)
    rpb = np.transpose(rpb, (2, 0, 1))                      # (h, q, k)
    # reorder heads to (r, hh) col layout: col block index g = r*2 + hh ->
    # we store free layout (r, hh, k): head h = hh*4 + r
    ord_heads = np.empty((4, 2, NTOK, NTOK), np.float32)
    for r in range(4):
        for hh in range(2):
            ord_heads[r, hh] = rpb[hh * 4 + r]
    # (q, r, hh, k)
    rpb_q = np.transpose(ord_heads, (2, 0, 1, 3)).reshape(NTOK, 4 * 2 * NTOK)
    if not mask_nonzero:
        e = np.exp(rpb_q)                                   # (64, 512)
        e2 = np.concatenate([e, e], axis=0)                 # (128, 512)
        return _np_bf16(e2), None
    # general path: per window-type table exp(rpb + mask[t])
    mask = np.asarray(inputs["mask"], np.float32)           # (49, 64, 64)
    ef = np.empty((NTOK, NW49, 512), np.float32)
    rq = rpb_q.reshape(NTOK, 4, 2, NTOK)
    for t in range(NW49):
        ef[:, t, :] = np.exp(rq + mask[t][:, None, None, :]).reshape(NTOK, 512)
    return None, _np_bf16(ef)


# ---- device kernel builder -------------------------------------------------
def _build(mask_nonzero, qkvb_nonzero, projb_nonzero, nchunk=NCHUNK):
    nc = bacc.Bacc("TRN2", target_bir_lowering=False, debug=False)

    x_d = nc.dram_tensor("x", (2, 128, TPC), BF16, kind="ExternalInput")
    y_d = nc.dram_tensor("y", (TPC, DIM), F32, kind="ExternalOutput")
    wqk_d = nc.dram_tensor("wqk", (2, 128, 512), BF16, kind="ExternalInput")
    wv_d = nc.dram_tensor("wv", (2, 128, 256), BF16, kind="ExternalInput")
    pw_d = nc.dram_tensor("pw", (2, 128, 256), BF16, kind="ExternalInput")
    if mask_nonzero:
        e_d = nc.dram_tensor("etab", (NTOK, NW49, 512), BF16, kind="ExternalInput")
    else:
        e_d = nc.dram_tensor("etab", (128, 512), BF16, kind="ExternalInput")
    if qkvb_nonzero:
        qkb_d = nc.dram_tensor("qkb", (4, 128), F32, kind="ExternalInput")
        vb_d = nc.dram_tensor("vb", (128, 256), BF16, kind="ExternalInput")
    if projb_nonzero:
        yb_d = nc.dram_tensor("yb", (128, 256), F32, kind="ExternalInput")

    import os as _os
    _pb = [int(v) for v in _os.environ.get("KPOOLS", "2,2,2,2").split(",")]
    with tile.TileContext(nc) as tc:
        with (
            tc.tile_pool(name="setup", bufs=1) as setup,
            tc.tile_pool(name="xin", bufs=2) as xin,
            tc.tile_pool(name="xts", bufs=3) as xts,
            tc.tile_pool(name="qks", bufs=3) as qks,
            tc.tile_pool(name="vs", bufs=3) as vsp,
            tc.tile_pool(name="ps", bufs=3) as psp,
            tc.tile_pool(name="pts", bufs=5) as pts,
            tc.tile_pool(name="avs", bufs=3) as avs,
            tc.tile_pool(name="ys", bufs=3) as ysp,
            tc.tile_pool(name="dst", bufs=8) as dst,
            tc.tile_pool(name="pp_qk", bufs=_pb[0], space="PSUM") as pp_qk,
            tc.tile_pool(name="pp_sc", bufs=_pb[1], space="PSUM") as pp_sc,
            tc.tile_pool(name="pp_pt", bufs=_pb[2], space="PSUM") as pp_pt,
            tc.tile_pool(name="pp_av", bufs=_pb[3], space="PSUM") as pp_av,
        ):
            # ---- one-time setup ----
            ident = setup.tile([128, 128], BF16)
            make_identity(nc, ident)

            wqk = setup.tile([128, 2, 512], BF16)
            nc.gpsimd.dma_start(
                out=wqk,
                in_=bass.AP(tensor=wqk_d, offset=0,
                            ap=[[512, 128], [128 * 512, 2], [1, 512]]),
            )
            wv = setup.tile([128, 2, 256], BF16)
            nc.gpsimd.dma_start(
                out=wv,
                in_=bass.AP(tensor=wv_d, offset=0,
                            ap=[[256, 128], [128 * 256, 2], [1, 256]]),
            )
            pw = setup.tile([128, 2, 256], BF16)
            nc.gpsimd.dma_start(
                out=pw,
                in_=bass.AP(tensor=pw_d, offset=0,
                            ap=[[256, 128], [128 * 256, 2], [1, 256]]),
            )
            if mask_nonzero:
                etab = setup.tile([NTOK, NW49, 512], BF16)
                nc.gpsimd.dma_start(out=etab, in_=e_d.ap())
            else:
                etab = setup.tile([128, 512], BF16)
                nc.gpsimd.dma_start(out=etab, in_=e_d.ap())
            if qkvb_nonzero:
                qkb = setup.tile([128, 4], F32)
                nc.gpsimd.dma_start(
                    out=qkb,
                    in_=bass.AP(tensor=qkb_d, offset=0, ap=[[1, 128], [128, 4]]),
                )
                vb = setup.tile([128, 256], BF16)
                nc.gpsimd.dma_start(out=vb, in_=vb_d.ap())
            if projb_nonzero:
                yb = setup.tile([128, 256], F32)
                nc.gpsimd.dma_start(out=yb, in_=yb_d.ap())

            # ---- main loop over chunks of 8 windows (512 tokens) ----
            for c in range(nchunk):
                tok0 = c * 512

                # xT load (host pre-transposed, c-major): (128, 2 kk, 512)
                xt = xts.tile([128, 2, 512], BF16)
                nc.sync.dma_start(
                    out=xt,
                    in_=bass.AP(tensor=x_d, offset=tok0,
                                ap=[[TPC, 128], [128 * TPC, 2], [1, 512]]),
                )

                # qkT = Wqk.T @ xT : 4 m-tiles (q0 q1 k0 k1), psum (128,512)
                qk = qks.tile([128, 4, 512], BF16)
                for m in (0, 2, 1, 3):
                    qkp = pp_qk.tile([128, 512], F32, name="qkp", tag="mm")
                    for kk in range(2):
                        nc.tensor.matmul(
                            qkp,
                            wqk[:, kk, 128 * m:128 * (m + 1)],
                            xt[:, kk, :],
                            start=(kk == 0),
                            stop=(kk == 1),
                        )
                    if qkvb_nonzero:
                        if m < 2:
                            nc.scalar.activation(
                                qk[:, m, :], qkp, AF.Copy, bias=qkb[:, m:m + 1]
                            )
                        else:
                            nc.vector.tensor_scalar_add(
                                qk[:, m, :], qkp, qkb[:, m:m + 1]
                            )
                    else:
                        if m < 2:
                            nc.scalar.copy(qk[:, m, :], qkp)
                        else:
                            nc.vector.tensor_copy(qk[:, m, :], qkp)

                # v (token-major) per pair j: psum (128, 256)
                v_sb = vsp.tile([128, 4, 256], BF16)
                vd_sb = vsp.tile([128, 4, 256], BF16)
                for j in range(4):
                    vp = pp_qk.tile([128, 256], F32, name="vp", tag="mm")
                    for kk in range(2):
                        nc.tensor.matmul(
                            vp,
                            xt[:, kk, 128 * j:128 * (j + 1)],
                            wv[:, kk, :],
                            start=(kk == 0),
                            stop=(kk == 1),
                        )
                    if qkvb_nonzero:
                        nc.vector.tensor_tensor(v_sb[:, j, :], vp, vb, ALU.add)
                    else:
                        nc.scalar.copy(v_sb[:, j, :], vp)  # ACT


                # duplicate v with swapped 64-row halves (for head-parity)
                nc.gpsimd.dma_start(out=vd_sb[0:64], in_=v_sb[64:128])
                nc.gpsimd.dma_start(out=vd_sb[64:128], in_=v_sb[0:64])

                # scores: per-r 1-bank tiles, layout (j, hh, k); exp per r
                p_sb = psp.tile([128, 2048], BF16)
                for r in range(4):
                    sc = pp_sc.tile([128, 512], F32, name=f"sc{r}", tag="sc")
                    scv = sc.rearrange("p (j hh k) -> p j hh k", j=4, hh=2)
                    for j in range(4):
                        for hh in range(2):
                            for win in range(2):
                                nc.tensor.matmul(
                                    scv[64 * win:64 * (win + 1), j, hh, :],
                                    qk[32 * r:32 * r + 32, hh,
                                       128 * j + 64 * win:128 * j + 64 * win + 64],
                                    qk[32 * r:32 * r + 32, 2 + hh,
                                       128 * j + 64 * win:128 * j + 64 * win + 64],
                                    tile_position=(32 * r, 64 * win),
                                )
                    nc.scalar.activation(
                        p_sb[:, 512 * r:512 * (r + 1)], sc, AF.Exp
                    )
                    # fold in exp(rpb [+ mask]) and row-sums for this r
                    pvr = p_sb[:, 512 * r:512 * (r + 1)].rearrange(
                        "p (j hh k) -> p j hh k", j=4, hh=2)
                    if r == 0:
                        dsum = dst.tile([128, 32], F32)
                    if not mask_nonzero:
                        e_ap = bass.AP(
                            tensor=etab.tensor,
                            offset=etab.offset + 128 * r,
                            ap=[etab.ap[0], [0, 4], [64, 2], [1, 64]],
                        )
                        nc.vector.tensor_tensor(pvr, pvr, e_ap, ALU.mult)
                        nc.vector.tensor_reduce(
                            dsum[:, 8 * r:8 * (r + 1)],
                            p_sb[:, 512 * r:512 * (r + 1)].rearrange(
                                "p (g k) -> p g k", g=8),
                            axis=mybir.AxisListType.X, op=ALU.add,
                        )
                    else:
                        for w in range(CHW):
                            t49 = (c * CHW + w) % NW49
                            j, win = w // 2, w % 2
                            dst_ap = pvr[64 * win:64 * (win + 1), j, :, :]
                            e_ap = bass.AP(
                                tensor=etab.tensor,
                                offset=etab.offset + t49 * 512 + 128 * r,
                                ap=[etab.ap[0], [64, 2], [1, 64]],
                            )
                            nc.vector.tensor_tensor(dst_ap, dst_ap, e_ap, ALU.mult)
                        nc.vector.tensor_reduce(
                            dsum[:, 8 * r:8 * (r + 1)],
                            p_sb[:, 512 * r:512 * (r + 1)].rearrange(
                                "p (g k) -> p g k", g=8),
                            axis=mybir.AxisListType.X, op=ALU.add,
                        )

                # reciprocal + normalize (in r-halves)
                rec = dst.tile([128, 32], F32)
                nc.vector.reciprocal_approx_fast(rec, dsum)
                recb = dst.tile([128, 32], BF16)
                nc.vector.tensor_copy(recb, rec)
                for hf in range(2):
                    r_ap = bass.AP(
                        tensor=recb.tensor, offset=recb.offset + 16 * hf,
                        ap=[recb.ap[0], [1, 16], [0, 64]],
                    )
                    p3v = p_sb[:, 1024 * hf:1024 * (hf + 1)].rearrange(
                        "p (g k) -> p g k", g=16)
                    nc.vector.tensor_tensor(p3v, p3v, r_ap, ALU.mult)

                # per pair: transpose probs, evac, av matmuls
                avp = [
                    pp_av.tile([128, 512], F32, name=f"avp{hh}", tag="avp")
                    for hh in range(2)
                ]
                for j in range(4):
                    ptp = pp_pt.tile([128, 512], BF16, name="ptp", tag="pt")
                    for r in range(4):
                        nc.tensor.transpose(
                            ptp[:, 128 * r:128 * (r + 1)],
                            p_sb[:, 512 * r + 128 * j:512 * r + 128 * j + 128],
                            ident,
                        )
                    pt_sb = pts.tile([128, 512], BF16)
                    if j % 2 == 0:
                        nc.vector.tensor_copy(pt_sb, ptp)
                    else:
                        nc.scalar.copy(pt_sb, ptp)
                    for hh in range(2):
                        for win in range(2):
                            vt = v_sb if win == hh else vd_sb
                            for r in range(4):
                                h = hh * 4 + r
                                nc.tensor.matmul(
                                    avp[hh][32 * r:32 * r + 32,
                                            64 * (2 * j + win):64 * (2 * j + win) + 64],
                                    vt[64 * hh:64 * hh + 64, j,
                                       32 * h:32 * h + 32],
                                    pt_sb[64 * hh:64 * hh + 64,
                                          128 * r + 64 * win:128 * r + 64 * win + 64],
                                    tile_position=(64 * hh, 32 * r),
                                )

                # evac av (feat-major out) then proj
                av_sb = avs.tile([128, 2, 512], BF16)
                nc.scalar.copy(av_sb[:, 0, :], avp[0])
                nc.scalar.copy(av_sb[:, 1, :], avp[1])

                y_sb = ysp.tile([128, 4, 256], F32)
                for j in range(4):
                    yp = pp_av.tile([128, 256], F32, name="yp", tag="avp")
                    for kk in range(2):
                        nc.tensor.matmul(
                            yp,
                            av_sb[:, kk, 128 * j:128 * (j + 1)],
                            pw[:, kk, :],
                            start=(kk == 0),
                            stop=(kk == 1),
                        )
                    if projb_nonzero:
                        nc.vector.tensor_tensor(y_sb[:, j, :], yp, yb, ALU.add)
                    else:
                        if j < 2:
                            nc.vector.tensor_copy(y_sb[:, j, :], yp)
                        else:
                            nc.scalar.copy(y_sb[:, j, :], yp)

                nc.sync.dma_start(
                    out=bass.AP(tensor=y_d, offset=tok0 * DIM,
                                ap=[[DIM, 128], [128 * DIM, 4], [1, DIM]]),
                    in_=y_sb,
                )

    nc.compile()
    return nc


# ---- execution --------------------------------------------------------------
_CACHE = {}


def _get_runner(mask_nonzero, qkvb_nonzero, projb_nonzero, nchunk=NCHUNK):
    key = (mask_nonzero, qkvb_nonzero, projb_nonzero, nchunk)
    if key in _CACHE:
        return _CACHE[key]

    nc = _build(mask_nonzero, qkvb_nonzero, projb_nonzero, nchunk)

    import jax
    import jax.numpy as jnp
    from jax.sharding import Mesh, PartitionSpec
    from jax.experimental.shard_map import shard_map
    from concourse import bass2jax
    from concourse.bass2jax import _bass_exec_p, install_neuronx_cc_hook

    install_neuronx_cc_hook()

    partition_name = (
        nc.partition_id_tensor.name if nc.partition_id_tensor else None
    )
    in_names, out_names, out_avals, zero_outs = [], [], [], []
    for alloc in nc.m.functions[0].allocations:
        if not isinstance(alloc, mybir.MemoryLocationSet):
            continue
        name = alloc.memorylocations[0].name
        if alloc.kind == "ExternalInput":
            if name != partition_name:
                in_names.append(name)
        elif alloc.kind == "ExternalOutput":
            shape = tuple(alloc.tensor_shape)
            dtype = mybir.dt.np(alloc.dtype)
            out_names.append(name)
            out_avals.append(jax.core.ShapedArray(shape, dtype))
            zero_outs.append(np.zeros(shape, dtype))
    n_params = len(in_names)
    n_outs = len(out_avals)
    all_in_names = list(in_names) + list(out_names)
    if partition_name is not None:
        all_in_names.append(partition_name)

    def _body(*args):
        operands = list(args)
        if partition_name is not None:
            operands.append(bass2jax.partition_id_tensor())
        outs = _bass_exec_p.bind(
            *operands,
            out_avals=tuple(out_avals),
            in_names=tuple(all_in_names),
            out_names=tuple(out_names),
            lowering_input_output_aliases=(),
            sim_require_finite=True,
            sim_require_nnan=True,
            nc=nc,
        )
        return tuple(outs)

    devices = jax.devices()[:NCORES]
    mesh = Mesh(np.asarray(devices), ("core",))
    donate = tuple(range(n_params, n_params + n_outs))
    sharded = jax.jit(
        shard_map(
            _body, mesh=mesh,
            in_specs=(PartitionSpec("core"),) * (n_params + n_outs),
            out_specs=(PartitionSpec("core"),) * n_outs,
            check_rep=False,
        ),
        donate_argnums=donate,
        keep_unused=True,
    )

    from jax.sharding import NamedSharding

    shard = NamedSharding(mesh, PartitionSpec("core"))
    zero_shapes = [
        ((NCORES * z.shape[0], *z.shape[1:]), z.dtype) for z in zero_outs
    ]
    make_zeros = jax.jit(
        lambda: tuple(jnp.zeros(s, d) for s, d in zero_shapes),
        out_shardings=(shard,) * n_outs,
    )

    def _concat(in_maps):
        return [
            np.concatenate([np.asarray(in_maps[c][nm]) for c in range(NCORES)], axis=0)
            for nm in in_names
        ]

    def run(in_maps):
        out_arrs = sharded(*_concat(in_maps), *make_zeros())
        out = np.asarray(out_arrs[out_names.index("y")])
        return out.reshape(NCORES, TPC, DIM)

    def bench(in_maps, iters=8):
        import time as _time

        dev_in = [jax.device_put(a, shard) for a in _concat(in_maps)]
        jax.block_until_ready(dev_in)
        outs = sharded(*dev_in, *make_zeros())
        jax.block_until_ready(outs)
        ts = []
        for _ in range(iters):
            t0 = _time.perf_counter()
            outs = sharded(*dev_in, *make_zeros())
            jax.block_until_ready(outs)
            ts.append(_time.perf_counter() - t0)
        return min(ts), ts

    def bench_repeat(in_maps, reps, iters=6):
        """Async-chain `reps` dispatches (output ping-pongs into the donated
        slot) and block once; median over iters."""
        import time as _time

        dev_in = [jax.device_put(a, shard) for a in _concat(in_maps)]
        jax.block_until_ready(dev_in)
        outs = sharded(*dev_in, *make_zeros())
        jax.block_until_ready(outs)
        ts = []
        for _ in range(iters):
            outs = sharded(*dev_in, *make_zeros())
            jax.block_until_ready(outs)
            t0 = _time.perf_counter()
            for _ in range(reps):
                outs = sharded(*dev_in, *outs)
            jax.block_until_ready(outs)
            ts.append(_time.perf_counter() - t0)
        ts.sort()
        return ts[len(ts) // 2], ts

    run.bench = bench
    run.bench_repeat = bench_repeat
    _CACHE[key] = run
    return run


def _prep_in_maps(inputs):
    x = np.asarray(inputs["x"], np.float32).reshape(B_ * NTOK, DIM)
    qkv_w = np.asarray(inputs["qkv_w"], np.float32)
    qkv_b = np.asarray(inputs["qkv_b"], np.float32)
    proj_w = np.asarray(inputs["proj_w"], np.float32)
    proj_b = np.asarray(inputs["proj_b"], np.float32)
    mask_nonzero = bool(np.any(np.asarray(inputs["mask"]) != 0))
    qkvb_nonzero = bool(np.any(qkv_b != 0))
    projb_nonzero = bool(np.any(proj_b != 0))

    wqk_f = qkv_w[:512].copy()
    wqk_f[:256] *= SCALE                       # fold q scale into Wq
    wqk = _np_bf16(wqk_f.T.reshape(2, 128, 512))
    wv = _np_bf16(qkv_w[512:].T.reshape(2, 128, 256))
    pw = _np_bf16(proj_w.T.reshape(2, 128, 256))

    e2, efull = _host_dpb_table(inputs, mask_nonzero)
    etab = efull if mask_nonzero else e2

    shared = {"wqk": wqk, "wv": wv, "pw": pw, "etab": etab}
    if qkvb_nonzero:
        qkb_f = qkv_b[:512].copy()
        qkb_f[:256] *= SCALE
        shared["qkb"] = qkb_f.reshape(4, 128).astype(np.float32)
        shared["vb"] = _np_bf16(np.broadcast_to(qkv_b[512:], (128, 256)).copy())
    if projb_nonzero:
        shared["yb"] = np.broadcast_to(proj_b, (128, 256)).copy().astype(np.float32)

    in_maps = []
    for c in range(NCORES):
        m = dict(shared)
        xs = x[c * TPC:(c + 1) * TPC]
        m["x"] = np.ascontiguousarray(_np_bf16(xs.T)).reshape(2, 128, TPC)
        in_maps.append(m)
    flags = (mask_nonzero, qkvb_nonzero, projb_nonzero)
    return in_maps, flags


def kernel(**inputs) -> np.ndarray:
    in_maps, flags = _prep_in_maps(inputs)
    run = _get_runner(*flags)
    out = run(in_maps)                          # (8, TPC, DIM) f32
    return out.reshape(B_, NTOK, DIM)
